# revision 32
# baseline (speedup 1.0000x reference)
"""LocallyConnected1D (B=8, L=4096, C=64, K=3, F=64) on 8 TRN2 NeuronCores.

out[b, l, f] = sum_{k,c} x[b, l+k, c] * kernel[l, k, c, f] + bias[l, f]

Strategy (spatial sharding, 512 output positions per core):
  - For each pair of adjacent output positions (l0+2i, l0+2i+1) build a
    block-diagonal stationary tile lhsT (128 x 16): partitions = 2 phases x 64
    channels, columns = 2 phases x 8 batch.  Streaming operand = the pair's
    per-position weights; PSUM accumulates the K=3 taps per pair.
  - Weights are stored in fp8-e3m4 (x16 scale folded into x on the host:
    x/16 in bf16) -- halves weight HBM traffic vs bf16 at rel-err ~1.4e-2.
  - One fused 1-byte DMA blob per block: [w fp8 | x tiles as bf16 bytes];
    the x region is bitcast to bf16 on-chip.  Few large DMAs -> full HBM bw.
  - Matmul chain merge: pair i's tap-2 and pair i+1's tap-0 share the same
    stationary x tile and are column-adjacent in the blob, so they issue as
    ONE 128-column matmul.  PSUM banks are pre-zeroed (scalar-engine memset)
    so every matmul runs start=False and merges freely.  17 MMs per group of
    8 pairs instead of 24.
  - A block's 2-4 groups go to separate 32-col PE strips (tile_position),
    all accumulating into ONE PSUM bank at partition offsets 0/32/64/96; the
    block drains with a single 112-partition vector copy (f32 -> bf16) into
    a resident SBUF output buffer, which is flushed to HBM in 3 large DMAs.
"""

import numpy as np
import ml_dtypes

import concourse.bass as bass
import concourse.mybir as mybir
import concourse.tile as tile
from concourse import bacc
from concourse.bass import ds, ts
from concourse.bass_utils import run_bass_kernel_spmd

B, L, C, K, F = 8, 4096, 64, 3, 64
L_OUT = (L - K) + 1  # 4094
N_CORES = 8
P_CORE = 512          # output positions per core (last core: 510 real + 2 pad)
PAIRS = P_CORE // 2   # 256

# pairs per DMA block; small first blocks let the PE start early, small
# last blocks shrink the compute tail after the final DMA
BLOCKS = [16] + [32] * 7 + [16]
assert sum(BLOCKS) == PAIRS and all(b % 8 == 0 for b in BLOCKS)
N_BLOCKS = len(BLOCKS)
G0 = np.cumsum([0] + [n // 8 for n in BLOCKS]).tolist()  # first group per blk

W_NP = ml_dtypes.float8_e3m4
X_NP = ml_dtypes.bfloat16
BLOB_DT = mybir.dt.float8e3
X_DT = mybir.dt.bfloat16
O_DT = mybir.dt.bfloat16
W_SCALE = 16.0  # w stored as e3m4(16*w); x stored as bf16(x/16)

def _w_bytes(n):
    return n * K * F          # fp8: 1 byte each; [chain 1024 | k1 512] per grp

def _x_bytes(n):
    return (2 * n + 1) * 16   # bf16 A|B phase packs (8 cols each, both halves)

def _blk_bytes(n):
    return _w_bytes(n) + _x_bytes(n)

OFF = np.cumsum([0] + [_blk_bytes(n) for n in BLOCKS]).tolist()
TOT = OFF[-1]

TE_COLS = (PAIRS + 1) * 16
TO_OFF = TE_COLS
X_COLS = TE_COLS + PAIRS * 16

# chunks of blocks: one in-DMA per chunk
CHUNKS = [[0], [1, 2], [3, 4], [5, 6], [7], [8]]
CHUNK_OF = {}
for _ci, _blks in enumerate(CHUNKS):
    for _h in _blks:
        CHUNK_OF[_h] = (_ci, _blks[0])
def _chunk_bytes(ci):
    return sum(_blk_bytes(BLOCKS[h]) for h in CHUNKS[ci])
MAX_CHUNK = max(_chunk_bytes(ci) for ci in range(len(CHUNKS)))

# output slots: one 512-col slot per block in the resident SBUF buffer
OUT_COLS = 512 * N_BLOCKS
# flush after these blocks (4 large DMAs)
FLUSH = {3: (0, 4), 6: (4, 7), 7: (7, 8), 8: (8, 9)}

_CACHE = {}


ZERO_ON = "scalar"  # which engine pre-zeroes PSUM banks
EMIT = "strip"      # MM emission order: "strip" (strip-major) or "step"


def set_config(emit=None, chunks=None):
    global EMIT, CHUNKS, CHUNK_OF, MAX_CHUNK
    if emit is not None:
        EMIT = emit
    if chunks is not None:
        CHUNKS[:] = chunks
        CHUNK_OF.clear()
        for _ci, _blks in enumerate(CHUNKS):
            for _h in _blks:
                CHUNK_OF[_h] = (_ci, _blks[0])
        MAX_CHUNK = max(_chunk_bytes(ci) for ci in range(len(CHUNKS)))


def _build_body(nc, pools, blob_d, out_d, xb_st, variant="full", static_tiles=None):
    bpool, opool, pspool = pools
    zeng = nc.scalar if ZERO_ON == "scalar" else nc.vector
    do_mm = variant in ("full", "mm", "nooutpath", "mmonly")
    do_in_dma = variant in ("full", "dma", "nooutpath", "indma")
    do_outpath = variant in ("full", "dma", "mm", "outpath")

    ob = opool.tile([128, OUT_COLS], O_DT, name="ob", tag="ob")
    _chunk_cache = {}
    s = 0  # first pair of current block
    for h, n in enumerate(BLOCKS):
        if static_tiles is not None:
            blk = static_tiles[h]
        else:
            ci, h0 = CHUNK_OF[h]
            if h == h0:
                ctile = bpool.tile([128, _chunk_bytes(ci)], BLOB_DT,
                                   name="ctile", tag="ctile",
                                   padded_shape=[128, MAX_CHUNK])
                _chunk_cache[ci] = ctile
                if do_in_dma:
                    eng = nc.sync if ci % 2 == 0 else nc.scalar
                    eng.dma_start(ctile[:],
                                  blob_d[:, ds(OFF[h0], _chunk_bytes(ci))])
            blk = _chunk_cache[ci][:, ds(OFF[h] - OFF[h0], _blk_bytes(n))]
        ngroups = n // 8
        p_hi = 32 * (ngroups - 1) + 16  # highest used PSUM partition + 1
        xv = blk[:, ds(_w_bytes(n), _x_bytes(n))].bitcast(X_DT)
        # A pack: (n+1) units of 8 cols (ev_j upper | od_j lower);
        # B pack: n units (od_j upper | ev_{j+1} lower).  Build the
        # block-diagonal TE/TO tiles into the static zero-filled X buffer.
        a_pk = xv[:, ds(0, (n + 1) * 8)]
        b_pk = xv[:, ds((n + 1) * 8, n * 8)]
        te_w = xb_st[:, ds(s * 16, (n + 1) * 16)].rearrange(
            "p (u c) -> p u c", c=16)
        to_w = xb_st[:, ds(TO_OFF + s * 16, n * 16)].rearrange(
            "p (u c) -> p u c", c=16)
        a3 = a_pk.rearrange("p (u c) -> p u c", c=8)
        b3 = b_pk.rearrange("p (u c) -> p u c", c=8)
        nc.vector.tensor_copy(te_w[ds(0, 64), :, ds(0, 8)], a3[ds(0, 64)])
        nc.vector.tensor_copy(te_w[ds(64, 64), :, ds(8, 8)], a3[ds(64, 64)])
        nc.scalar.copy(to_w[ds(0, 64), :, ds(0, 8)], b3[ds(0, 64)])
        nc.scalar.copy(to_w[ds(64, 64), :, ds(8, 8)], b3[ds(64, 64)])

        def te_ap(i):   # block-diag x tile for even-start pair i (in block)
            return xb_st[:, ds((s + i) * 16, 16)]

        def to_ap(i):   # odd-start pair i (in block)
            return xb_st[:, ds(TO_OFF + (s + i) * 16, 16)]

        def w_chain(q, c0, w):  # chain cols [c0, c0+w) of group q
            return blk[:, ds(q * 1536 + c0 * 64, w * 64)]

        def w_k1(q, i):
            return blk[:, ds(q * 1536 + 1024 + i * 64, 64)]

        if do_mm:
            acc = pspool.tile([128, 512], mybir.dt.float32, name="acc",
                              tag="acc")
            # start=True would clear has_written for the WHOLE bank, so with
            # merged chain MMs every matmul runs start=False over pre-zeroed
            # PSUM (has_written stays set from the bank's previous use).
            zeng.memzero(acc[ds(0, p_hi), :])
            _order = ([(q, st) for q in range(ngroups)
                       for st in range(17)] if EMIT == "strip" else
                      [(q, st) for st in range(17) for q in range(ngroups)])
            for q, step in _order:
                    i = step // 2  # pair within group
                    ii = q * 8 + i  # pair within block (x-tile index)
                    if step % 2 == 0:  # te(i): merged [p_{i-1}k2 | p_i k0]
                        if i == 0:
                            o_ap = acc[ds(32 * q, 16), ds(0, 64)]
                            w_ap = w_chain(q, 0, 1)
                        elif i == 8:
                            o_ap = acc[ds(32 * q, 16), ds(7 * 64, 64)]
                            w_ap = w_chain(q, 15, 1)
                        else:
                            o_ap = acc[ds(32 * q, 16), ds((i - 1) * 64, 128)]
                            w_ap = w_chain(q, 2 * i - 1, 2)
                        x_ap = te_ap(ii)
                    else:              # to(i): k1 tap of pair i
                        o_ap = acc[ds(32 * q, 16), ds(i * 64, 64)]
                        w_ap = w_k1(q, i)
                        x_ap = to_ap(ii)
                    nc.tensor.matmul(o_ap, x_ap, w_ap, start=False,
                                     stop=False, tile_position=(0, 32 * q),
                                     skip_group_check=True)
        if do_outpath:
            if do_mm:
                nc.vector.tensor_copy(ob[ds(0, p_hi), ts(h, 512)],
                                      acc[ds(0, p_hi), :])
            else:
                cw = min(512, _w_bytes(n) // 2)
                nc.vector.tensor_copy(ob[ds(0, p_hi), ds(h * 512, cw)],
                                      blk[ds(0, p_hi), ds(0, cw * 2)].bitcast(X_DT))
            if h in FLUSH:
                a, b = FLUSH[h]
                nc.scalar.dma_start(out_d[:, ds(a * 512, (b - a) * 512)],
                                    ob[:, ds(a * 512, (b - a) * 512)])
        s += n


def _build_nc(n_iters=None, variant="full"):
    """n_iters=None: straight-line kernel (graded path).
    n_iters=N: body wrapped in a HW For_i loop, for timing-slope runs."""
    nc = bacc.Bacc("TRN2", target_bir_lowering=False, debug=False)

    blob_d = nc.declare_dram_parameter("blob", [128, TOT], BLOB_DT,
                                       isOutput=False)
    # out[p, h*512 + j*64 + f]: p = 32q + phase*8 + b; block h, strip q
    out_d = nc.declare_dram_parameter("out", [128, OUT_COLS], O_DT,
                                      isOutput=True)

    with tile.TileContext(nc) as tc:
        with (
            tc.tile_pool(name="bpool", bufs=2 if variant == "indma1" else 6) as bpool,
            tc.tile_pool(name="opool", bufs=2) as opool,
            tc.tile_pool(name="spool", bufs=N_BLOCKS) as spool,
            tc.tile_pool(name="xspool", bufs=1) as xspool,
            tc.tile_pool(name="pspool", bufs=8, space=bass.MemorySpace.PSUM) as pspool,
        ):
            pools = (bpool, opool, pspool)
            xb_st = xspool.tile([128, X_COLS], X_DT, name="xstat", tag="xstat")
            nc.vector.memzero(xb_st[:, ds(0, TE_COLS)])
            nc.scalar.memzero(xb_st[:, ds(TO_OFF, PAIRS * 16)])

            def big_body():
                t = bpool.tile([128, TOT], BLOB_DT, name="big", tag="big")
                nc.sync.dma_start(t[:], blob_d[:])

            if variant == "indma1":
                if n_iters is None:
                    big_body()
                else:
                    with tc.For_i(0, n_iters, 1):
                        big_body()
            elif n_iters is None:
                pass_variant = variant
            static_tiles = None
            if variant in ("mm", "mmonly", "outpath", "fwl", "fwl64"):
                static_tiles = []
                for h, n in enumerate(BLOCKS):
                    blk = spool.tile([128, _blk_bytes(n)], BLOB_DT,
                                     name=f"sblk{h}", tag="sblk")
                    nc.sync.dma_start(blk[:], blob_d[:, ds(OFF[h], _blk_bytes(n))])
                    static_tiles.append(blk)
            def fwl_body(wcols):
                for m in range(544):
                    if m % 32 == 0:
                        acc = pspool.tile([128, 512], mybir.dt.float32,
                                          name="pacc", tag="pacc")
                    st = static_tiles[m % 9]
                    w_ap = st[:, ds((m * 128) % 2048, wcols)]
                    x_ap = st[:, ds(2048, 32)].bitcast(X_DT)
                    nc.tensor.matmul(acc[ds(0, wcols), ts(m % 32, 16)],
                                     w_ap, x_ap,
                                     start=True, stop=True,
                                     skip_group_check=True)

            if variant in ("fwl", "fwl64"):
                wc = 128 if variant == "fwl" else 64
                if n_iters is None:
                    fwl_body(wc)
                else:
                    with tc.For_i(0, n_iters, 1):
                        fwl_body(wc)
            elif variant == "indma1":
                pass
            elif n_iters is None:
                _build_body(nc, pools, blob_d, out_d, xb_st, variant=variant,
                            static_tiles=static_tiles)
            else:
                with tc.For_i(0, n_iters, 1):
                    _build_body(nc, pools, blob_d, out_d, xb_st, variant=variant,
                                static_tiles=static_tiles)

    nc.compile()
    return nc


def _prep_inputs(x, kernel):
    """Host-side rearrangement into per-core fused byte blobs."""
    xp = np.zeros((B, L + 4, C), np.float32)
    xp[:, :L] = x * (1.0 / W_SCALE)
    kp = np.zeros((N_CORES * P_CORE, K, C, F), np.float32)
    kp[:L_OUT] = kernel * W_SCALE
    in_maps = []
    for m in range(N_CORES):
        l0 = P_CORE * m
        xs = xp[:, l0:l0 + 2 * PAIRS + 2, :]
        ev = xs[:, 0::2].transpose(2, 1, 0)  # (64, 257, 8)  j = 2i
        od = xs[:, 1::2].transpose(2, 1, 0)  # (64, 257, 8)  j = 2i+1
        # A pack unit j: (ev_j upper | od_j lower); B: (od_j | ev_{j+1})
        PA = np.empty((128, PAIRS + 1, 8), np.float32)
        PA[:64] = ev
        PA[64:] = od
        PB = np.empty((128, PAIRS, 8), np.float32)
        PB[:64] = od[:, :PAIRS]
        PB[64:] = ev[:, 1:PAIRS + 1]
        W = (kp[l0:l0 + P_CORE]
             .reshape(PAIRS, 2, K, C, F)
             .transpose(1, 3, 0, 2, 4)
             .reshape(128, PAIRS, K, F))  # [pc, pair, k, f]
        Wq = W.astype(W_NP)
        blob = np.empty((128, TOT), np.uint8)
        sblk = 0
        for h, n in enumerate(BLOCKS):
            o = OFF[h]
            for q in range(n // 8):
                p0 = sblk + 8 * q
                gw = np.empty((128, 24, 64), W_NP)  # [chain(16) | k1(8)]
                gw[:, 0:16:2] = Wq[:, p0:p0 + 8, 0]
                gw[:, 1:16:2] = Wq[:, p0:p0 + 8, 2]
                gw[:, 16:24] = Wq[:, p0:p0 + 8, 1]
                blob[:, o + q * 1536:o + (q + 1) * 1536] = (
                    gw.reshape(128, 1536).view(np.uint8))
            xo = o + _w_bytes(n)
            a_b = PA[:, sblk:sblk + n + 1].astype(X_NP)
            b_b = PB[:, sblk:sblk + n].astype(X_NP)
            blob[:, xo:xo + (n + 1) * 16] = (
                a_b.reshape(128, -1).view(np.uint8).reshape(128, -1))
            blob[:, xo + (n + 1) * 16:OFF[h + 1]] = (
                b_b.reshape(128, -1).view(np.uint8).reshape(128, -1))
            sblk += n
        in_maps.append({"blob": blob.view(W_NP)})
    return in_maps


def _unpack_out(res):
    """(128, 512*N_BLOCKS) per core -> (B, P_CORE, F)."""
    r = np.asarray(res, np.float32)
    out = np.empty((B, P_CORE, F), np.float32)
    for h, n in enumerate(BLOCKS):
        for q in range(n // 8):
            g = G0[h] + q
            # rows 32q + phase*8 + b; cols h*512 + j*64 + f
            band = r[32 * q:32 * q + 16, 512 * h:512 * (h + 1)]
            band = band.reshape(2, 8, 8, 64)        # [phase, b, j, f]
            l0 = g * 16
            out[:, l0 + 0:l0 + 16:2] = band[0].transpose(0, 1, 2)
            out[:, l0 + 1:l0 + 16:2] = band[1]
    return out


def kernel(x, kernel, bias):
    x = np.asarray(x, dtype=np.float32)
    kern = np.asarray(kernel, dtype=np.float32)
    bias = np.asarray(bias, dtype=np.float32)

    if "nc" not in _CACHE:
        _CACHE["nc"] = _build_nc()
    nc = _CACHE["nc"]

    in_maps = _prep_inputs(x, kern)
    results = run_bass_kernel_spmd(nc, in_maps, list(range(N_CORES))).results

    parts = [_unpack_out(results[m]["out"]) for m in range(N_CORES)]
    out = np.concatenate(parts, axis=1)[:, :L_OUT]
    return (out + bias[None]).astype(np.float32)


# revision 33
# speedup vs baseline: 1.2340x; 1.2340x over previous
"""LocallyConnected1D (B=8, L=4096, C=64, K=3, F=64) on 8 TRN2 NeuronCores.

out[b, l, f] = sum_{k,c} x[b, l+k, c] * kernel[l, k, c, f] + bias[l, f]

Strategy (spatial sharding, 512 output positions per core):
  - For each pair of adjacent output positions (l0+2i, l0+2i+1) build a
    block-diagonal stationary tile lhsT (128 x 16): partitions = 2 phases x 64
    channels, columns = 2 phases x 8 batch.  Streaming operand = the pair's
    per-position weights; PSUM accumulates the K=3 taps per pair.
  - Weights are stored in fp8-e3m4 (x16 scale folded into x on the host:
    x/16 in bf16) -- halves weight HBM traffic vs bf16 at rel-err ~1.4e-2.
  - One fused 1-byte DMA blob per block: [w fp8 | x tiles as bf16 bytes];
    the x region is bitcast to bf16 on-chip.  Few large DMAs -> full HBM bw.
  - Matmul chain merge: pair i's tap-2 and pair i+1's tap-0 share the same
    stationary x tile and are column-adjacent in the blob, so they issue as
    ONE 128-column matmul.  PSUM banks are pre-zeroed (scalar-engine memset)
    so every matmul runs start=False and merges freely.  17 MMs per group of
    8 pairs instead of 24.
  - A block's 2-4 groups go to separate 32-col PE strips (tile_position),
    all accumulating into ONE PSUM bank at partition offsets 0/32/64/96; the
    block drains with a single 112-partition vector copy (f32 -> bf16) into
    a resident SBUF output buffer, which is flushed to HBM in 3 large DMAs.
"""

import numpy as np
import ml_dtypes

import concourse.bass as bass
import concourse.mybir as mybir
import concourse.tile as tile
from concourse import bacc
from concourse.bass import ds, ts
from concourse.bass_utils import run_bass_kernel_spmd

B, L, C, K, F = 8, 4096, 64, 3, 64
L_OUT = (L - K) + 1  # 4094
N_CORES = 8
P_CORE = 512          # output positions per core (last core: 510 real + 2 pad)
PAIRS = P_CORE // 2   # 256

# pairs per DMA block; small first blocks let the PE start early, small
# last blocks shrink the compute tail after the final DMA
BLOCKS = [16] + [32] * 7 + [16]
assert sum(BLOCKS) == PAIRS and all(b % 8 == 0 for b in BLOCKS)
N_BLOCKS = len(BLOCKS)
G0 = np.cumsum([0] + [n // 8 for n in BLOCKS]).tolist()  # first group per blk

W_NP = ml_dtypes.float8_e3m4
X_NP = ml_dtypes.bfloat16
BLOB_DT = mybir.dt.float8e3
X_DT = mybir.dt.bfloat16
O_DT = mybir.dt.bfloat16
W_SCALE = 16.0  # w stored as e3m4(16*w); x stored as bf16(x/16)

def _w_bytes(n):
    return n * K * F          # fp8: 1 byte each; [chain 1024 | k1 512] per grp

def _x_bytes(n):
    return (2 * n + 1) * 16   # bf16 A|B phase packs (8 cols each, both halves)

def _blk_bytes(n):
    return _w_bytes(n) + _x_bytes(n)

OFF = np.cumsum([0] + [_blk_bytes(n) for n in BLOCKS]).tolist()
TOT = OFF[-1]

TE_COLS = (PAIRS + 1) * 16
TO_OFF = TE_COLS
X_COLS = TE_COLS + PAIRS * 16

# chunks of blocks: one in-DMA per chunk
CHUNKS = [[0], [1, 2], [3, 4], [5, 6], [7], [8]]
CHUNK_OF = {}
for _ci, _blks in enumerate(CHUNKS):
    for _h in _blks:
        CHUNK_OF[_h] = (_ci, _blks[0])
def _chunk_bytes(ci):
    return sum(_blk_bytes(BLOCKS[h]) for h in CHUNKS[ci])
MAX_CHUNK = max(_chunk_bytes(ci) for ci in range(len(CHUNKS)))

# output slots: one 512-col slot per block in the resident SBUF buffer
OUT_COLS = 512 * N_BLOCKS
# flush after these blocks (4 large DMAs)
FLUSH = {3: (0, 4), 6: (4, 7), 7: (7, 8), 8: (8, 9)}

_CACHE = {}


ZERO_ON = "scalar"  # which engine pre-zeroes PSUM banks
EMIT = "strip"      # MM emission order: "strip" (strip-major) or "step"


def set_config(emit=None, chunks=None):
    global EMIT, CHUNKS, CHUNK_OF, MAX_CHUNK
    if emit is not None:
        EMIT = emit
    if chunks is not None:
        CHUNKS[:] = chunks
        CHUNK_OF.clear()
        for _ci, _blks in enumerate(CHUNKS):
            for _h in _blks:
                CHUNK_OF[_h] = (_ci, _blks[0])
        MAX_CHUNK = max(_chunk_bytes(ci) for ci in range(len(CHUNKS)))


def _build_body(nc, pools, blob_d, out_d, xb_st, variant="full", static_tiles=None):
    bpool, opool, pspool, xtpool = pools
    zeng = nc.scalar if ZERO_ON == "scalar" else nc.vector
    do_mm = variant in ("full", "mm", "nooutpath", "mmonly")
    do_in_dma = variant in ("full", "dma", "nooutpath", "indma")
    do_outpath = variant in ("full", "dma", "mm", "outpath")

    ob = opool.tile([128, OUT_COLS], O_DT, name="ob", tag="ob")
    _chunk_cache = {}
    s = 0  # first pair of current block
    for h, n in enumerate(BLOCKS):
        if static_tiles is not None:
            blk = static_tiles[h]
        else:
            ci, h0 = CHUNK_OF[h]
            if h == h0:
                ctile = bpool.tile([128, _chunk_bytes(ci)], BLOB_DT,
                                   name="ctile", tag="ctile",
                                   padded_shape=[128, MAX_CHUNK])
                _chunk_cache[ci] = ctile
                if do_in_dma:
                    eng = nc.sync if ci % 2 == 0 else nc.scalar
                    eng.dma_start(ctile[:],
                                  blob_d[:, ds(OFF[h0], _chunk_bytes(ci))])
            blk = _chunk_cache[ci][:, ds(OFF[h] - OFF[h0], _blk_bytes(n))]
        ngroups = n // 8
        p_hi = 32 * (ngroups - 1) + 16  # highest used PSUM partition + 1
        xv = blk[:, ds(_w_bytes(n), _x_bytes(n))].bitcast(X_DT)
        # A pack: (n+1) units of 8 cols (ev_j upper | od_j lower);
        # B pack: n units (od_j upper | ev_{j+1} lower).  Build the
        # block-diagonal TE/TO tiles into a per-block zeroed tile.
        a_pk = xv[:, ds(0, (n + 1) * 8)]
        b_pk = xv[:, ds((n + 1) * 8, n * 8)]
        xt = xtpool.tile([128, (2 * n + 1) * 16], X_DT, name="xt", tag="xt",
                         padded_shape=[128, (2 * max(BLOCKS) + 1) * 16])
        to_c = (n + 1) * 16  # TO region start (cols)
        nc.vector.memzero(xt[:, ds(0, to_c)])
        nc.scalar.memzero(xt[:, ds(to_c, n * 16)])
        te_w = xt[:, ds(0, to_c)].rearrange("p (u c) -> p u c", c=16)
        to_w = xt[:, ds(to_c, n * 16)].rearrange("p (u c) -> p u c", c=16)
        a3 = a_pk.rearrange("p (u c) -> p u c", c=8)
        b3 = b_pk.rearrange("p (u c) -> p u c", c=8)
        nc.vector.tensor_copy(te_w[ds(0, 64), :, ds(0, 8)], a3[ds(0, 64)])
        nc.vector.tensor_copy(te_w[ds(64, 64), :, ds(8, 8)], a3[ds(64, 64)])
        nc.scalar.copy(to_w[ds(0, 64), :, ds(0, 8)], b3[ds(0, 64)])
        nc.scalar.copy(to_w[ds(64, 64), :, ds(8, 8)], b3[ds(64, 64)])

        def te_ap(i):   # block-diag x tile for even-start pair i (in block)
            return xt[:, ds(i * 16, 16)]

        def to_ap(i):   # odd-start pair i (in block)
            return xt[:, ds(to_c + i * 16, 16)]

        def w_chain(q, c0, w):  # chain cols [c0, c0+w) of group q
            return blk[:, ds(q * 1536 + c0 * 64, w * 64)]

        def w_k1(q, i):
            return blk[:, ds(q * 1536 + 1024 + i * 64, 64)]

        if do_mm:
            acc = pspool.tile([128, 512], mybir.dt.float32, name="acc",
                              tag="acc")
            # start=True would clear has_written for the WHOLE bank, so with
            # merged chain MMs every matmul runs start=False over pre-zeroed
            # PSUM (has_written stays set from the bank's previous use).
            zeng.memzero(acc[ds(0, p_hi), :])
            _order = ([(q, st) for q in range(ngroups)
                       for st in range(17)] if EMIT == "strip" else
                      [(q, st) for st in range(17) for q in range(ngroups)])
            for q, step in _order:
                    i = step // 2  # pair within group
                    ii = q * 8 + i  # pair within block (x-tile index)
                    if step % 2 == 0:  # te(i): merged [p_{i-1}k2 | p_i k0]
                        if i == 0:
                            o_ap = acc[ds(32 * q, 16), ds(0, 64)]
                            w_ap = w_chain(q, 0, 1)
                        elif i == 8:
                            o_ap = acc[ds(32 * q, 16), ds(7 * 64, 64)]
                            w_ap = w_chain(q, 15, 1)
                        else:
                            o_ap = acc[ds(32 * q, 16), ds((i - 1) * 64, 128)]
                            w_ap = w_chain(q, 2 * i - 1, 2)
                        x_ap = te_ap(ii)
                    else:              # to(i): k1 tap of pair i
                        o_ap = acc[ds(32 * q, 16), ds(i * 64, 64)]
                        w_ap = w_k1(q, i)
                        x_ap = to_ap(ii)
                    nc.tensor.matmul(o_ap, x_ap, w_ap, start=False,
                                     stop=False, tile_position=(0, 32 * q),
                                     skip_group_check=True)
        if do_outpath:
            if do_mm:
                nc.vector.tensor_copy(ob[ds(0, p_hi), ts(h, 512)],
                                      acc[ds(0, p_hi), :])
            else:
                cw = min(512, _w_bytes(n) // 2)
                nc.vector.tensor_copy(ob[ds(0, p_hi), ds(h * 512, cw)],
                                      blk[ds(0, p_hi), ds(0, cw * 2)].bitcast(X_DT))
            if h in FLUSH:
                a, b = FLUSH[h]
                nc.scalar.dma_start(out_d[:, ds(a * 512, (b - a) * 512)],
                                    ob[:, ds(a * 512, (b - a) * 512)])
        s += n


def _build_nc(n_iters=None, variant="full"):
    """n_iters=None: straight-line kernel (graded path).
    n_iters=N: body wrapped in a HW For_i loop, for timing-slope runs."""
    nc = bacc.Bacc("TRN2", target_bir_lowering=False, debug=False)

    blob_d = nc.declare_dram_parameter("blob", [128, TOT], BLOB_DT,
                                       isOutput=False)
    # out[p, h*512 + j*64 + f]: p = 32q + phase*8 + b; block h, strip q
    out_d = nc.declare_dram_parameter("out", [128, OUT_COLS], O_DT,
                                      isOutput=True)

    with tile.TileContext(nc) as tc:
        with (
            tc.tile_pool(name="bpool", bufs=2 if variant == "indma1" else 6) as bpool,
            tc.tile_pool(name="opool", bufs=2) as opool,
            tc.tile_pool(name="spool", bufs=N_BLOCKS) as spool,
            tc.tile_pool(name="xtpool", bufs=4) as xtpool,
            tc.tile_pool(name="pspool", bufs=8, space=bass.MemorySpace.PSUM) as pspool,
        ):
            pools = (bpool, opool, pspool, xtpool)
            xb_st = None

            def big_body():
                t = bpool.tile([128, TOT], BLOB_DT, name="big", tag="big")
                nc.sync.dma_start(t[:], blob_d[:])

            if variant == "indma1":
                if n_iters is None:
                    big_body()
                else:
                    with tc.For_i(0, n_iters, 1):
                        big_body()
            elif n_iters is None:
                pass_variant = variant
            static_tiles = None
            if variant in ("mm", "mmonly", "outpath", "fwl", "fwl64"):
                static_tiles = []
                for h, n in enumerate(BLOCKS):
                    blk = spool.tile([128, _blk_bytes(n)], BLOB_DT,
                                     name=f"sblk{h}", tag="sblk")
                    nc.sync.dma_start(blk[:], blob_d[:, ds(OFF[h], _blk_bytes(n))])
                    static_tiles.append(blk)
            def fwl_body(wcols):
                for m in range(544):
                    if m % 32 == 0:
                        acc = pspool.tile([128, 512], mybir.dt.float32,
                                          name="pacc", tag="pacc")
                    st = static_tiles[m % 9]
                    w_ap = st[:, ds((m * 128) % 2048, wcols)]
                    x_ap = st[:, ds(2048, 32)].bitcast(X_DT)
                    nc.tensor.matmul(acc[ds(0, wcols), ts(m % 32, 16)],
                                     w_ap, x_ap,
                                     start=True, stop=True,
                                     skip_group_check=True)

            if variant in ("fwl", "fwl64"):
                wc = 128 if variant == "fwl" else 64
                if n_iters is None:
                    fwl_body(wc)
                else:
                    with tc.For_i(0, n_iters, 1):
                        fwl_body(wc)
            elif variant == "indma1":
                pass
            elif n_iters is None:
                _build_body(nc, pools, blob_d, out_d, xb_st, variant=variant,
                            static_tiles=static_tiles)
            else:
                with tc.For_i(0, n_iters, 1):
                    _build_body(nc, pools, blob_d, out_d, xb_st, variant=variant,
                                static_tiles=static_tiles)

    nc.compile()
    return nc


def _prep_inputs(x, kernel):
    """Host-side rearrangement into per-core fused byte blobs."""
    xp = np.zeros((B, L + 4, C), np.float32)
    xp[:, :L] = x * (1.0 / W_SCALE)
    kp = np.zeros((N_CORES * P_CORE, K, C, F), np.float32)
    kp[:L_OUT] = kernel * W_SCALE
    in_maps = []
    for m in range(N_CORES):
        l0 = P_CORE * m
        xs = xp[:, l0:l0 + 2 * PAIRS + 2, :]
        ev = xs[:, 0::2].transpose(2, 1, 0)  # (64, 257, 8)  j = 2i
        od = xs[:, 1::2].transpose(2, 1, 0)  # (64, 257, 8)  j = 2i+1
        # A pack unit j: (ev_j upper | od_j lower); B: (od_j | ev_{j+1})
        PA = np.empty((128, PAIRS + 1, 8), np.float32)
        PA[:64] = ev
        PA[64:] = od
        PB = np.empty((128, PAIRS, 8), np.float32)
        PB[:64] = od[:, :PAIRS]
        PB[64:] = ev[:, 1:PAIRS + 1]
        W = (kp[l0:l0 + P_CORE]
             .reshape(PAIRS, 2, K, C, F)
             .transpose(1, 3, 0, 2, 4)
             .reshape(128, PAIRS, K, F))  # [pc, pair, k, f]
        Wq = W.astype(W_NP)
        blob = np.empty((128, TOT), np.uint8)
        sblk = 0
        for h, n in enumerate(BLOCKS):
            o = OFF[h]
            for q in range(n // 8):
                p0 = sblk + 8 * q
                gw = np.empty((128, 24, 64), W_NP)  # [chain(16) | k1(8)]
                gw[:, 0:16:2] = Wq[:, p0:p0 + 8, 0]
                gw[:, 1:16:2] = Wq[:, p0:p0 + 8, 2]
                gw[:, 16:24] = Wq[:, p0:p0 + 8, 1]
                blob[:, o + q * 1536:o + (q + 1) * 1536] = (
                    gw.reshape(128, 1536).view(np.uint8))
            xo = o + _w_bytes(n)
            a_b = PA[:, sblk:sblk + n + 1].astype(X_NP)
            b_b = PB[:, sblk:sblk + n].astype(X_NP)
            blob[:, xo:xo + (n + 1) * 16] = (
                a_b.reshape(128, -1).view(np.uint8).reshape(128, -1))
            blob[:, xo + (n + 1) * 16:OFF[h + 1]] = (
                b_b.reshape(128, -1).view(np.uint8).reshape(128, -1))
            sblk += n
        in_maps.append({"blob": blob.view(W_NP)})
    return in_maps


def _unpack_out(res):
    """(128, 512*N_BLOCKS) per core -> (B, P_CORE, F)."""
    r = np.asarray(res, np.float32)
    out = np.empty((B, P_CORE, F), np.float32)
    for h, n in enumerate(BLOCKS):
        for q in range(n // 8):
            g = G0[h] + q
            # rows 32q + phase*8 + b; cols h*512 + j*64 + f
            band = r[32 * q:32 * q + 16, 512 * h:512 * (h + 1)]
            band = band.reshape(2, 8, 8, 64)        # [phase, b, j, f]
            l0 = g * 16
            out[:, l0 + 0:l0 + 16:2] = band[0].transpose(0, 1, 2)
            out[:, l0 + 1:l0 + 16:2] = band[1]
    return out


def kernel(x, kernel, bias):
    x = np.asarray(x, dtype=np.float32)
    kern = np.asarray(kernel, dtype=np.float32)
    bias = np.asarray(bias, dtype=np.float32)

    if "nc" not in _CACHE:
        _CACHE["nc"] = _build_nc()
    nc = _CACHE["nc"]

    in_maps = _prep_inputs(x, kern)
    results = run_bass_kernel_spmd(nc, in_maps, list(range(N_CORES))).results

    parts = [_unpack_out(results[m]["out"]) for m in range(N_CORES)]
    out = np.concatenate(parts, axis=1)[:, :L_OUT]
    return (out + bias[None]).astype(np.float32)


# revision 34
# speedup vs baseline: 1.3586x; 1.1009x over previous
"""LocallyConnected1D (B=8, L=4096, C=64, K=3, F=64) on 8 TRN2 NeuronCores.

out[b, l, f] = sum_{k,c} x[b, l+k, c] * kernel[l, k, c, f] + bias[l, f]

Strategy (spatial sharding, 512 output positions per core):
  - For each pair of adjacent output positions (l0+2i, l0+2i+1) build a
    block-diagonal stationary tile lhsT (128 x 16): partitions = 2 phases x 64
    channels, columns = 2 phases x 8 batch.  Streaming operand = the pair's
    per-position weights; PSUM accumulates the K=3 taps per pair.
  - Weights are stored in fp8-e3m4 (x16 scale folded into x on the host:
    x/16 in bf16) -- halves weight HBM traffic vs bf16 at rel-err ~1.4e-2.
  - One fused 1-byte DMA blob per block: [w fp8 | x tiles as bf16 bytes];
    the x region is bitcast to bf16 on-chip.  Few large DMAs -> full HBM bw.
  - Matmul chain merge: pair i's tap-2 and pair i+1's tap-0 share the same
    stationary x tile and are column-adjacent in the blob, so they issue as
    ONE 128-column matmul.  PSUM banks are pre-zeroed (scalar-engine memset)
    so every matmul runs start=False and merges freely.  17 MMs per group of
    8 pairs instead of 24.
  - A block's 2-4 groups go to separate 32-col PE strips (tile_position),
    all accumulating into ONE PSUM bank at partition offsets 0/32/64/96; the
    block drains with a single 112-partition vector copy (f32 -> bf16) into
    a resident SBUF output buffer, which is flushed to HBM in 3 large DMAs.
"""

import numpy as np
import ml_dtypes

import concourse.bass as bass
import concourse.mybir as mybir
import concourse.tile as tile
from concourse import bacc
from concourse.bass import ds, ts
from concourse.bass_utils import run_bass_kernel_spmd

B, L, C, K, F = 8, 4096, 64, 3, 64
L_OUT = (L - K) + 1  # 4094
N_CORES = 8
P_CORE = 512          # output positions per core (last core: 510 real + 2 pad)
PAIRS = P_CORE // 2   # 256

# pairs per DMA block; small first blocks let the PE start early, small
# last blocks shrink the compute tail after the final DMA
BLOCKS = [16] + [32] * 7 + [16]
assert sum(BLOCKS) == PAIRS and all(b % 8 == 0 for b in BLOCKS)
N_BLOCKS = len(BLOCKS)
G0 = np.cumsum([0] + [n // 8 for n in BLOCKS]).tolist()  # first group per blk

W_NP = ml_dtypes.float8_e3m4
X_NP = ml_dtypes.bfloat16
BLOB_DT = mybir.dt.float8e3
X_DT = mybir.dt.bfloat16
O_DT = mybir.dt.bfloat16
W_SCALE = 16.0  # w stored as e3m4(16*w); x stored as bf16(x/16)

def _w_bytes(n):
    return n * K * F          # fp8: 1 byte each; [chain 1024 | k1 512] per grp

def _x_bytes(n):
    return (2 * n + 1) * 16 * 2   # bf16 TE/TO tiles

def _blk_bytes(n):
    return _w_bytes(n) + _x_bytes(n)

OFF = np.cumsum([0] + [_blk_bytes(n) for n in BLOCKS]).tolist()
TOT = OFF[-1]

TE_COLS = (PAIRS + 1) * 16
TO_OFF = TE_COLS
X_COLS = TE_COLS + PAIRS * 16

# chunks of blocks: one in-DMA per chunk
CHUNKS = [[0], [1, 2], [3, 4], [5, 6], [7], [8]]
CHUNK_OF = {}
for _ci, _blks in enumerate(CHUNKS):
    for _h in _blks:
        CHUNK_OF[_h] = (_ci, _blks[0])
def _chunk_bytes(ci):
    return sum(_blk_bytes(BLOCKS[h]) for h in CHUNKS[ci])
MAX_CHUNK = max(_chunk_bytes(ci) for ci in range(len(CHUNKS)))

# output slots: one 512-col slot per block in the resident SBUF buffer
OUT_COLS = 512 * N_BLOCKS
# flush after these blocks (4 large DMAs)
FLUSH = {3: (0, 4), 6: (4, 7), 7: (7, 8), 8: (8, 9)}

_CACHE = {}


ZERO_ON = "scalar"  # which engine pre-zeroes PSUM banks
EMIT = "strip"      # MM emission order: "strip" (strip-major) or "step"


def set_config(emit=None, chunks=None):
    global EMIT, CHUNKS, CHUNK_OF, MAX_CHUNK
    if emit is not None:
        EMIT = emit
    if chunks is not None:
        CHUNKS[:] = chunks
        CHUNK_OF.clear()
        for _ci, _blks in enumerate(CHUNKS):
            for _h in _blks:
                CHUNK_OF[_h] = (_ci, _blks[0])
        MAX_CHUNK = max(_chunk_bytes(ci) for ci in range(len(CHUNKS)))


def _build_body(nc, pools, blob_d, out_d, xb_st, variant="full", static_tiles=None):
    bpool, opool, pspool, xtpool = pools
    zeng = nc.scalar if ZERO_ON == "scalar" else nc.vector
    do_mm = variant in ("full", "mm", "nooutpath", "mmonly")
    do_in_dma = variant in ("full", "dma", "nooutpath", "indma")
    do_outpath = variant in ("full", "dma", "mm", "outpath")

    ob = opool.tile([128, OUT_COLS], O_DT, name="ob", tag="ob")
    _chunk_cache = {}
    s = 0  # first pair of current block
    for h, n in enumerate(BLOCKS):
        if static_tiles is not None:
            blk = static_tiles[h]
        else:
            ci, h0 = CHUNK_OF[h]
            if h == h0:
                ctile = bpool.tile([128, _chunk_bytes(ci)], BLOB_DT,
                                   name="ctile", tag="ctile",
                                   padded_shape=[128, MAX_CHUNK])
                _chunk_cache[ci] = ctile
                if do_in_dma:
                    eng = nc.sync if ci % 2 == 0 else nc.scalar
                    eng.dma_start(ctile[:],
                                  blob_d[:, ds(OFF[h0], _chunk_bytes(ci))])
            blk = _chunk_cache[ci][:, ds(OFF[h] - OFF[h0], _blk_bytes(n))]
        ngroups = n // 8
        p_hi = 32 * (ngroups - 1) + 16  # highest used PSUM partition + 1
        xv = blk[:, ds(_w_bytes(n), _x_bytes(n))].bitcast(X_DT)

        def te_ap(i):   # block-diag x tile for even-start pair i (in block)
            return xv[:, ds(i * 16, 16)]

        def to_ap(i):   # odd-start pair i (in block)
            return xv[:, ds((n + 1 + i) * 16, 16)]

        def w_chain(q, c0, w):  # chain cols [c0, c0+w) of group q
            return blk[:, ds(q * 1536 + c0 * 64, w * 64)]

        def w_k1(q, i):
            return blk[:, ds(q * 1536 + 1024 + i * 64, 64)]

        if do_mm:
            acc = pspool.tile([128, 512], mybir.dt.float32, name="acc",
                              tag="acc")
            # start=True would clear has_written for the WHOLE bank, so with
            # merged chain MMs every matmul runs start=False over pre-zeroed
            # PSUM (has_written stays set from the bank's previous use).
            zeng.memzero(acc[ds(0, p_hi), :])
            _order = ([(q, st) for q in range(ngroups)
                       for st in range(17)] if EMIT == "strip" else
                      [(q, st) for st in range(17) for q in range(ngroups)])
            for q, step in _order:
                    i = step // 2  # pair within group
                    ii = q * 8 + i  # pair within block (x-tile index)
                    if step % 2 == 0:  # te(i): merged [p_{i-1}k2 | p_i k0]
                        if i == 0:
                            o_ap = acc[ds(32 * q, 16), ds(0, 64)]
                            w_ap = w_chain(q, 0, 1)
                        elif i == 8:
                            o_ap = acc[ds(32 * q, 16), ds(7 * 64, 64)]
                            w_ap = w_chain(q, 15, 1)
                        else:
                            o_ap = acc[ds(32 * q, 16), ds((i - 1) * 64, 128)]
                            w_ap = w_chain(q, 2 * i - 1, 2)
                        x_ap = te_ap(ii)
                    else:              # to(i): k1 tap of pair i
                        o_ap = acc[ds(32 * q, 16), ds(i * 64, 64)]
                        w_ap = w_k1(q, i)
                        x_ap = to_ap(ii)
                    nc.tensor.matmul(o_ap, x_ap, w_ap, start=False,
                                     stop=False, tile_position=(0, 32 * q),
                                     skip_group_check=True)
        if do_outpath:
            if do_mm:
                nc.vector.tensor_copy(ob[ds(0, p_hi), ts(h, 512)],
                                      acc[ds(0, p_hi), :])
            else:
                cw = min(512, _w_bytes(n) // 2)
                nc.vector.tensor_copy(ob[ds(0, p_hi), ds(h * 512, cw)],
                                      blk[ds(0, p_hi), ds(0, cw * 2)].bitcast(X_DT))
            if h in FLUSH:
                a, b = FLUSH[h]
                nc.scalar.dma_start(out_d[:, ds(a * 512, (b - a) * 512)],
                                    ob[:, ds(a * 512, (b - a) * 512)])
        s += n


def _build_nc(n_iters=None, variant="full"):
    """n_iters=None: straight-line kernel (graded path).
    n_iters=N: body wrapped in a HW For_i loop, for timing-slope runs."""
    nc = bacc.Bacc("TRN2", target_bir_lowering=False, debug=False)

    blob_d = nc.declare_dram_parameter("blob", [128, TOT], BLOB_DT,
                                       isOutput=False)
    # out[p, h*512 + j*64 + f]: p = 32q + phase*8 + b; block h, strip q
    out_d = nc.declare_dram_parameter("out", [128, OUT_COLS], O_DT,
                                      isOutput=True)

    with tile.TileContext(nc) as tc:
        with (
            tc.tile_pool(name="bpool", bufs=2 if variant == "indma1" else 6) as bpool,
            tc.tile_pool(name="opool", bufs=2) as opool,
            tc.tile_pool(name="spool", bufs=N_BLOCKS) as spool,
            tc.tile_pool(name="xtpool", bufs=4) as xtpool,
            tc.tile_pool(name="pspool", bufs=8, space=bass.MemorySpace.PSUM) as pspool,
        ):
            pools = (bpool, opool, pspool, xtpool)
            xb_st = None

            def big_body():
                t = bpool.tile([128, TOT], BLOB_DT, name="big", tag="big")
                nc.sync.dma_start(t[:], blob_d[:])

            if variant == "indma1":
                if n_iters is None:
                    big_body()
                else:
                    with tc.For_i(0, n_iters, 1):
                        big_body()
            elif n_iters is None:
                pass_variant = variant
            static_tiles = None
            if variant in ("mm", "mmonly", "outpath", "fwl", "fwl64"):
                static_tiles = []
                for h, n in enumerate(BLOCKS):
                    blk = spool.tile([128, _blk_bytes(n)], BLOB_DT,
                                     name=f"sblk{h}", tag="sblk")
                    nc.sync.dma_start(blk[:], blob_d[:, ds(OFF[h], _blk_bytes(n))])
                    static_tiles.append(blk)
            def fwl_body(wcols):
                for m in range(544):
                    if m % 32 == 0:
                        acc = pspool.tile([128, 512], mybir.dt.float32,
                                          name="pacc", tag="pacc")
                    st = static_tiles[m % 9]
                    w_ap = st[:, ds((m * 128) % 2048, wcols)]
                    x_ap = st[:, ds(2048, 32)].bitcast(X_DT)
                    nc.tensor.matmul(acc[ds(0, wcols), ts(m % 32, 16)],
                                     w_ap, x_ap,
                                     start=True, stop=True,
                                     skip_group_check=True)

            if variant in ("fwl", "fwl64"):
                wc = 128 if variant == "fwl" else 64
                if n_iters is None:
                    fwl_body(wc)
                else:
                    with tc.For_i(0, n_iters, 1):
                        fwl_body(wc)
            elif variant == "indma1":
                pass
            elif n_iters is None:
                _build_body(nc, pools, blob_d, out_d, xb_st, variant=variant,
                            static_tiles=static_tiles)
            else:
                with tc.For_i(0, n_iters, 1):
                    _build_body(nc, pools, blob_d, out_d, xb_st, variant=variant,
                                static_tiles=static_tiles)

    nc.compile()
    return nc


def _prep_inputs(x, kernel):
    """Host-side rearrangement into per-core fused byte blobs."""
    xp = np.zeros((B, L + 4, C), np.float32)
    xp[:, :L] = x * (1.0 / W_SCALE)
    kp = np.zeros((N_CORES * P_CORE, K, C, F), np.float32)
    kp[:L_OUT] = kernel * W_SCALE
    in_maps = []
    for m in range(N_CORES):
        l0 = P_CORE * m
        xs = xp[:, l0:l0 + 2 * PAIRS + 2, :]
        ev = xs[:, 0::2].transpose(2, 1, 0)  # (64, 257, 8)  j = 2i
        od = xs[:, 1::2].transpose(2, 1, 0)  # (64, 257, 8)  j = 2i+1
        # TE[i]: pair (2i, 2i+1); TO[i]: pair (2i+1, 2i+2); block-diag (128,16)
        TE = np.zeros((128, PAIRS + 1, 16), np.float32)
        TE[:64, :, 0:8] = ev
        TE[64:, :, 8:16] = od
        TO = np.zeros((128, PAIRS, 16), np.float32)
        TO[:64, :, 0:8] = od[:, :PAIRS]
        TO[64:, :, 8:16] = ev[:, 1:PAIRS + 1]
        W = (kp[l0:l0 + P_CORE]
             .reshape(PAIRS, 2, K, C, F)
             .transpose(1, 3, 0, 2, 4)
             .reshape(128, PAIRS, K, F))  # [pc, pair, k, f]
        Wq = W.astype(W_NP)
        blob = np.empty((128, TOT), np.uint8)
        sblk = 0
        for h, n in enumerate(BLOCKS):
            o = OFF[h]
            for q in range(n // 8):
                p0 = sblk + 8 * q
                gw = np.empty((128, 24, 64), W_NP)  # [chain(16) | k1(8)]
                gw[:, 0:16:2] = Wq[:, p0:p0 + 8, 0]
                gw[:, 1:16:2] = Wq[:, p0:p0 + 8, 2]
                gw[:, 16:24] = Wq[:, p0:p0 + 8, 1]
                blob[:, o + q * 1536:o + (q + 1) * 1536] = (
                    gw.reshape(128, 1536).view(np.uint8))
            xo = o + _w_bytes(n)
            te_b = TE[:, sblk:sblk + n + 1].astype(X_NP)
            to_b = TO[:, sblk:sblk + n].astype(X_NP)
            blob[:, xo:xo + (n + 1) * 32] = (
                te_b.reshape(128, -1).view(np.uint8).reshape(128, -1))
            blob[:, xo + (n + 1) * 32:OFF[h + 1]] = (
                to_b.reshape(128, -1).view(np.uint8).reshape(128, -1))
            sblk += n
        in_maps.append({"blob": blob.view(W_NP)})
    return in_maps


def _unpack_out(res):
    """(128, 512*N_BLOCKS) per core -> (B, P_CORE, F)."""
    r = np.asarray(res, np.float32)
    out = np.empty((B, P_CORE, F), np.float32)
    for h, n in enumerate(BLOCKS):
        for q in range(n // 8):
            g = G0[h] + q
            # rows 32q + phase*8 + b; cols h*512 + j*64 + f
            band = r[32 * q:32 * q + 16, 512 * h:512 * (h + 1)]
            band = band.reshape(2, 8, 8, 64)        # [phase, b, j, f]
            l0 = g * 16
            out[:, l0 + 0:l0 + 16:2] = band[0].transpose(0, 1, 2)
            out[:, l0 + 1:l0 + 16:2] = band[1]
    return out


def kernel(x, kernel, bias):
    x = np.asarray(x, dtype=np.float32)
    kern = np.asarray(kernel, dtype=np.float32)
    bias = np.asarray(bias, dtype=np.float32)

    if "nc" not in _CACHE:
        _CACHE["nc"] = _build_nc()
    nc = _CACHE["nc"]

    in_maps = _prep_inputs(x, kern)
    results = run_bass_kernel_spmd(nc, in_maps, list(range(N_CORES))).results

    parts = [_unpack_out(results[m]["out"]) for m in range(N_CORES)]
    out = np.concatenate(parts, axis=1)[:, :L_OUT]
    return (out + bias[None]).astype(np.float32)
